# revision 41
# baseline (speedup 1.0000x reference)
"""GAT (2-layer, 8-head) Trainium2 Bass kernel, 8-core SPMD. v2.

Strategy (dst-sharded edge partition, superwindows of 128 dsts):
- Host: append self-loops, shard edges by dst range (6250 dsts/core), bucket
  into 49 superwindows of 128 dsts, split each window's edges by src<32768
  (lo/hi for int16 dma_gather indexing). Sections padded to 128-edge chunks
  with SPMD-uniform (max-over-cores) chunk counts; pad slots carry idx=-1
  (skipped by the gather HW when num_idxs_reg = per-core valid count) and
  all-zero rows/cols in the one-hot S/ST matrices.
- Device phase 1: sharded matmul xT @ W1ext -> h rows
  [h(256) | a_src(8) | a_dst(8) | pad] bf16; a_dst slice kept in SBUF
  (ad_all); AllGather of the h table is CHUNKED (5 pieces) and overlaps the
  matmul tiles.
- Phase 2 (per superwindow): dma_gather h[src] rows (768B, Q7-bound, back to
  back); ed = ST-chunk @ a_dst matmuls; e = a_src[src]+ed (vector);
  w = max(exp(e), exp(0.2e)) (scalar engine exps, vector max) written into
  the gathered tile's a_dst cols; msg = h*w; one fused matmul per chunk
  accumulates aggregation + softmax denominators in PSUM; epilogue computes
  act1'=elu+1 via exp/relu on the scalar engine, transposes it, and fuses
  the layer-2 matmul (W2ext, with the -1 correction folded as a replicated
  constant row) -> h2 rows [h2(10)|as2|ad2|0...] written to h2_bounce;
  AllGather-2 is chunked behind the window loop.
- Phase 4: same edge pipeline with 1 head, 10 channels on 256B h2 rows ->
  final [6250, 10] fp32 slice per core; host concatenates.
"""
import os
import sys
from contextlib import ExitStack

for _p in ("/opt/trn_rl_repo", os.path.expanduser("~/.axon_site/_ro/trn_rl_repo")):
    if os.path.isdir(_p) and _p not in sys.path:
        sys.path.insert(0, _p)

import numpy as np
import ml_dtypes

P = 128


class Cfg:
    def __init__(self):
        self.N, self.F, self.HEADS, self.CH, self.NCLS = 50000, 767, 8, 32, 10
        self.NCORES, self.SPLIT, self.NEG = 8, 30000, 0.2
        self.HID = self.HEADS * self.CH            # 256
        self.DPC = self.N // self.NCORES           # 6250 dsts per core
        self.NW = (self.DPC + P - 1) // P          # 49 superwindows
        self.DPCP = self.NW * P                    # 6272 padded rows/core
        self.FP = (self.F + P - 1) // P * P        # 768
        self.KC1 = self.FP // P                    # 6
        self.W1C = self.HID + 2 * self.HEADS       # 272 used cols
        self.T1 = 384                              # 768B table rows
        self.KC2 = self.HID // P                   # 2
        self.W2C = self.NCLS + 2                   # 12 used cols
        self.T2 = 128                              # 256B table rows
        self.AGC = 5                               # AllGather chunks
        assert self.DPC % self.AGC == 0
        self.AGR = self.DPC // self.AGC            # 1250 rows/core/chunk


def _wrap_idxs(vals, nslots):
    """int16 vals (len<=nslots) -> [128, nslots/16] wrapped, pads=-1."""
    cols = nslots // 16
    arr = np.full((16, cols), -1, dtype=np.int16)
    n = len(vals)
    if n:
        j = np.arange(n)
        arr[j % 16, j // 16] = vals
    return np.tile(arr, (8, 1))


def preprocess(cfg, x, edge_index, W1, att_src1, att_dst1, b1, W2, att_src2,
               att_dst2, b2):
    c = cfg
    N = c.N
    # self-loops are handled locally in the epilogues, not as edges
    src = np.asarray(edge_index[0]).astype(np.int64)
    dst = np.asarray(edge_index[1]).astype(np.int64)

    # --- weight prep (param folding only) ---
    W1 = np.asarray(W1, np.float32)
    a_s1 = np.asarray(att_src1, np.float32)
    a_d1 = np.asarray(att_dst1, np.float32)
    W1e = np.zeros((c.FP, c.T1), np.float32)
    W1e[: c.F, : c.HID] = W1
    for h in range(c.HEADS):
        blk = W1[:, h * c.CH: (h + 1) * c.CH]
        W1e[: c.F, c.HID + h] = blk @ a_s1[h]
        W1e[: c.F, c.HID + c.HEADS + h] = blk @ a_d1[h]
    W2 = np.asarray(W2, np.float32)
    W2e = np.zeros((c.HID, c.T2), np.float32)
    W2e[:, : c.NCLS] = W2
    W2e[:, c.NCLS] = W2 @ np.asarray(att_src2, np.float32)[0]
    W2e[:, c.NCLS + 1] = W2 @ np.asarray(att_dst2, np.float32)[0]
    # act1 is stored as elu+1; fold the -1 row correction into a replicated
    # constant added to every h2 row.
    negrow = -W2e.sum(axis=0)                       # [T2]
    negrow_rep = np.tile(negrow[None, :], (P, 1)).astype(np.float32)

    # --- per-core edge bucketing (numpy group-by) ---
    core = dst // c.DPC
    dloc = dst - core * c.DPC
    win = dloc // P
    dcol = dloc % P
    # Table rows are laid out (ag_chunk, core, row) so each chunked
    # AllGather writes a contiguous slab; remap gather indices to match.
    core_s = src // c.DPC
    rr = src - core_s * c.DPC
    psrc = ((rr // c.AGR) * (c.AGR * c.NCORES) + core_s * c.AGR
            + rr % c.AGR)
    src = psrc
    is_hi = (src >= c.SPLIT).astype(np.int64)
    sec = ((core * c.NW + win) * 2 + is_hi)
    order = np.argsort(sec, kind="stable")
    sec_s = sec[order]
    src_s = src[order]
    dcol_s = dcol[order]
    nsec = c.NCORES * c.NW * 2
    bounds = np.searchsorted(sec_s, np.arange(nsec + 1))
    cnts = (bounds[1:] - bounds[:-1]).reshape(c.NCORES, c.NW, 2)

    def nch(n):
        return max(1, (int(n) + P - 1) // P)

    LC = [nch(cnts[:, w, 0].max()) for w in range(c.NW)]
    HC = [nch(cnts[:, w, 1].max()) for w in range(c.NW)]
    # +1 self-loop identity chunk per window (filled on-device, no gather)
    TOTC = sum(LC) + sum(HC) + c.NW
    sec_c0 = []                                    # chunk offset per (w, kind)
    off = 0
    for w in range(c.NW):
        sec_c0.append((off, off + LC[w]))
        off += LC[w] + HC[w] + 1
    meta = {"LC": LC, "HC": HC, "TOTC": TOTC, "sec_c0": sec_c0,
            "b2_zero": not np.any(np.asarray(b2))}

    in_maps = []
    xf = np.asarray(x, np.float32)
    ar = np.arange(P)
    for co in range(c.NCORES):
        idx_parts = []
        cnt_arr = np.zeros((1, 2 * c.NW), np.int32)
        dstc = np.full((P, TOTC), -1, np.int32)    # slot -> dst col (-1=pad)
        for w in range(c.NW):
            for kind in (0, 1):
                s = (co * c.NW + w) * 2 + kind
                b0, b1_ = bounds[s], bounds[s + 1]
                vals = src_s[b0:b1_] - (c.SPLIT if kind else 0)
                dcs = dcol_s[b0:b1_]
                nck = LC[w] if kind == 0 else HC[w]
                cstart = sec_c0[w][kind]
                n = b1_ - b0
                if n == 0:
                    # keep >=1 valid idx so the gather is never empty
                    vals = np.zeros(1, np.int64)
                    dcs = np.full(1, -2, np.int64)  # no S entry
                    n = 1
                j = np.arange(n)
                dstc[j % P, cstart + j // P] = np.where(dcs >= 0, dcs, -1)
                idx_parts.append(_wrap_idxs(vals.astype(np.int16), nck * P))
                cnt_arr[0, 2 * w + kind] = n
            # self-loop identity chunk (no idx cols; filled on-device)
            wd = min(P, c.DPC - w * P)
            sc = sec_c0[w][0] + LC[w] + HC[w]
            dstc[0:wd, sc] = np.arange(wd)
            idx_parts.append(np.full((P, P // 16), -1, np.int16))
        idx_np = np.concatenate(idx_parts, axis=1)
        assert idx_np.shape[1] == TOTC * (P // 16)

        S_host = (dstc[:, :, None] == ar[None, None, :])
        ST_host = (dstc.T[None, :, :] == ar[:, None, None])   # [j, chunk, e]
        S_host = S_host.astype(ml_dtypes.float8_e4m3).reshape(P, TOTC * P)
        ST_host = ST_host.astype(ml_dtypes.float8_e4m3).reshape(P, TOTC * P)

        xT = np.zeros((c.FP, c.DPCP), ml_dtypes.bfloat16)
        xs = xf[co * c.DPC: (co + 1) * c.DPC]
        xT[: c.F, : c.DPC] = xs.T.astype(ml_dtypes.bfloat16)

        in_maps.append({
            "xT": xT,
            "W1e": W1e.astype(ml_dtypes.bfloat16),
            "W2e": W2e.astype(ml_dtypes.bfloat16),
            "negrow": negrow_rep,
            "idx": idx_np,
            "cnts": cnt_arr,
            "Sh": S_host,
            "STh": ST_host,
            "b2r": np.tile(np.asarray(b2, np.float32)[None, :], (P, 1)),
        })
    return meta, in_maps


def build_program(cfg, meta):
    import concourse.bacc as bacc
    import concourse.mybir as mybir
    import concourse.tile as tile
    from concourse.library_config import mlp
    from concourse.masks import make_identity

    c = cfg
    f32, bf16 = mybir.dt.float32, mybir.dt.bfloat16
    fp8 = mybir.dt.float8e4
    AT = mybir.ActivationFunctionType
    OP = mybir.AluOpType

    nc = bacc.Bacc("TRN2", target_bir_lowering=False, debug=False,
                   num_devices=c.NCORES, num_swdge_queues=4)
    TOTC = meta["TOTC"]
    LC, HC, sec_c0 = meta["LC"], meta["HC"], meta["sec_c0"]
    GCs = [LC[w] + HC[w] + 1 for w in range(c.NW)]

    xT_d = nc.dram_tensor("xT", [c.FP, c.DPCP], bf16, kind="ExternalInput")
    W1e_d = nc.dram_tensor("W1e", [c.FP, c.T1], bf16, kind="ExternalInput")
    W2e_d = nc.dram_tensor("W2e", [c.HID, c.T2], bf16, kind="ExternalInput")
    negrow_d = nc.dram_tensor("negrow", [P, c.T2], f32, kind="ExternalInput")
    idx_d = nc.dram_tensor("idx", [P, TOTC * (P // 16)], mybir.dt.int16,
                           kind="ExternalInput")
    cnts_d = nc.dram_tensor("cnts", [1, 2 * c.NW], mybir.dt.int32,
                            kind="ExternalInput")
    Sh_d = nc.dram_tensor("Sh", [P, TOTC * P], fp8, kind="ExternalInput")
    STh_d = nc.dram_tensor("STh", [P, TOTC * P], fp8, kind="ExternalInput")
    b2r_d = nc.dram_tensor("b2r", [P, c.NCLS], f32, kind="ExternalInput")
    out_d = nc.dram_tensor("out", [c.DPC, c.NCLS], f32, kind="ExternalOutput")

    _shared = "Shared" if c.NCORES > 4 else "Local"
    AGR = c.AGR                                  # rows per AllGather chunk
    AGW = AGR * c.NCORES                         # global rows per chunk
    LO_CH = c.SPLIT // AGW                       # chunks in the lo table (3)
    assert c.SPLIT % AGW == 0
    h_bounce = nc.dram_tensor("h_bounce", [c.DPCP, c.T1], bf16, kind="Internal")
    h_tab_lo = nc.dram_tensor("h_tab_lo", [c.SPLIT, c.T1], bf16,
                              kind="Internal", addr_space=_shared)
    h_tab_hi = nc.dram_tensor("h_tab_hi", [c.N - c.SPLIT, c.T1], bf16,
                              kind="Internal", addr_space=_shared)
    h2_bounce = nc.dram_tensor("h2_bounce", [c.DPCP, c.T2], bf16,
                               kind="Internal")
    h2_tab_lo = nc.dram_tensor("h2_tab_lo", [c.SPLIT, c.T2], bf16,
                               kind="Internal", addr_space=_shared)
    h2_tab_hi = nc.dram_tensor("h2_tab_hi", [c.N - c.SPLIT, c.T2], bf16,
                               kind="Internal", addr_space=_shared)

    ag_marks = {}
    for ci in range(c.AGC):
        ag_marks[min(((ci + 1) * AGR + P - 1) // P, c.NW) - 1] = ci
    groups8 = [list(range(c.NCORES))]

    def ag_chunk(ci, bounce, tab_lo, tab_hi):
        r0, r1 = ci * AGR, (ci + 1) * AGR
        tab, g0 = ((tab_lo, ci * AGW) if ci < LO_CH
                   else (tab_hi, ci * AGW - c.SPLIT))
        nc.gpsimd.collective_compute(
            "AllGather", OP.bypass, replica_groups=groups8,
            ins=[bounce.ap()[r0:r1, :]],
            outs=[tab.ap()[g0: g0 + AGW, :]])

    with ExitStack() as stack:
        tc = stack.enter_context(tile.TileContext(nc))
        cpool = stack.enter_context(tc.tile_pool(name="consts", bufs=1))
        nc.gpsimd.load_library(mlp)

        ident = cpool.tile([P, P], f32)
        make_identity(nc, ident[:])
        b2r_t = cpool.tile([P, c.NCLS], f32)
        nc.sync.dma_start(b2r_t[:], b2r_d[:])
        negrow_t = cpool.tile([P, c.T2], f32)
        nc.sync.dma_start(negrow_t[:], negrow_d[:])
        w2s = []
        for k in range(c.KC2):
            t = cpool.tile([P, c.T2], bf16, tag=f"w2s{k}")
            nc.sync.dma_start(t[:], W2e_d[k * P: (k + 1) * P, :])
            w2s.append(t)
        ad_all = cpool.tile([P, c.NW, c.HEADS], bf16, tag="ad_all")
        hown_all = cpool.tile([P, c.NW, c.HID + c.HEADS], bf16, tag="hown")
        h2own_all = cpool.tile([P, c.NW, c.W2C], bf16, tag="h2own")

        # ---------------- phase 1: L1 matmul (sharded rows) ----------------
        with tc.tile_pool(name="mm1", bufs=1) as mm1, \
             tc.tile_pool(name="mm1w", bufs=3) as mm1w, \
             tc.tile_pool(name="mm1p", bufs=2, space="PSUM") as mm1p:
            w1s = []
            for k in range(c.KC1):
                t = mm1.tile([P, c.T1], bf16, tag=f"w1s{k}")
                nc.sync.dma_start(t[:], W1e_d[k * P: (k + 1) * P, :])
                w1s.append(t)
            # xT loaded in column halves so matmuls start after ~half the load
            RH0 = (c.NW + 1) // 2                  # tiles in first half
            HW0 = RH0 * P
            xts = [[None, None] for _ in range(c.KC1)]
            for half in range(2):
                cw = HW0 if half == 0 else c.DPCP - HW0
                for k in range(c.KC1):
                    t = mm1.tile([P, cw], bf16, tag=f"xts{k}_{half}")
                    nc.sync.dma_start(
                        t[:], xT_d[k * P: (k + 1) * P,
                                   half * HW0: half * HW0 + cw])
                    xts[k][half] = t
            for r in range(c.NW):
                half, rh = (0, r) if r < RH0 else (1, r - RH0)
                ps = mm1p.tile([P, c.T1], f32, space="PSUM", tag="mmps")
                for k in range(c.KC1):
                    nc.tensor.matmul(
                        ps[:], lhsT=xts[k][half][:, rh * P: (rh + 1) * P],
                        rhs=w1s[k][:], start=(k == 0), stop=(k == c.KC1 - 1))
                hsb = mm1w.tile([P, c.T1], bf16, tag="hsb")
                nc.scalar.copy(hsb[:], ps[:])
                nc.vector.tensor_copy(
                    out=ad_all[:, r, :],
                    in_=hsb[:, c.HID + c.HEADS: c.HID + 2 * c.HEADS])
                nc.vector.tensor_copy(
                    out=hown_all[:, r, :],
                    in_=hsb[:, 0: c.HID + c.HEADS])
                nc.sync.dma_start(h_bounce[r * P: (r + 1) * P, :], hsb[:])
                if r in ag_marks:
                    ag_chunk(ag_marks[r], h_bounce, h_tab_lo, h_tab_hi)

        # ---------------- shared edge-window pipeline ----------------
        LEAD = 3

        def edge_phase(pools, tab_lo, tab_hi, ad_tile, own_tile, owncols,
                       adw, heads, ch, gelem, wcol, rhsw, epilogue):
            eg, ew, eS, ep1, ep2 = pools
            hc = heads * ch
            GCmax = max(GCs)
            qn = [0]
            cnt_reg = nc.gpsimd.alloc_register(f"cnt_reg_{id(epilogue)}")
            gts = {}

            def gather(gt, sec_n, sec_i, col_off, out_off, in_ap):
                nidx = sec_n * P
                nc.gpsimd.reg_load(
                    cnt_reg, cnt_t[0:1, sec_i: sec_i + 1])
                nc.gpsimd.dma_gather(
                    gt[:, out_off: out_off + sec_n, :], in_ap,
                    idx_t[:, col_off * (P // 16):
                          (col_off + sec_n) * (P // 16)],
                    nidx, cnt_reg, gelem, single_packet=False,
                    queue_num=qn[0] % 4)
                qn[0] += 1

            def issue_lo(w):
                GC = GCs[w]
                c0 = sec_c0[w][0]
                gtf = eg.tile([P, GCmax, gelem], bf16, tag="gt")
                if w < 2:
                    nc.vector.memset(gtf[:], 0.0)
                else:
                    nc.vector.memset(gtf[:, :, hc: hc + adw], 0.0)
                gt = gtf[:, 0:GC, :]
                gts[w] = gt
                gather(gt, LC[w], 2 * w, c0, 0, tab_lo.ap())

            def run_window(w):
                GC = GCs[w]
                c0 = sec_c0[w][0]
                gt = gts.pop(w)
                gather(gt, HC[w], 2 * w + 1, c0 + LC[w], LC[w],
                       tab_hi.ap())
                # self-loop chunk: own rows copied locally, S/ST = identity
                nc.vector.tensor_copy(
                    out=gt[:, GC - 1, 0: owncols],
                    in_=own_tile[:, w, 0: owncols])
                S_g = eS.tile([P, GC, P], fp8, tag="Sg")
                nc.sync.dma_start(S_g[:], Sh_d[:, c0 * P: (c0 + GC) * P])
                ST_g = eS.tile([P, GC, P], fp8, tag="STg")
                nc.sync.dma_start(ST_g[:], STh_d[:, c0 * P: (c0 + GC) * P])

                # ed = a_dst broadcast per edge slot
                ed_ps = ep1.tile([P, GC, adw], f32, space="PSUM", tag="edps")
                for k in range(GC):
                    nc.tensor.matmul(
                        ed_ps[:, k, :], lhsT=ST_g[:, k, :],
                        rhs=ad_tile[:, w, :], start=True, stop=True)
                # e = a_src + ed; w = max(exp(e), exp(0.2 e))
                e_t = ew.tile([P, GC, adw], f32, tag="e")
                nc.vector.tensor_tensor(
                    out=e_t[:], in0=gt[:, :, hc: hc + adw],
                    in1=ed_ps[:], op=OP.add)
                w1_t = ew.tile([P, GC, adw], f32, tag="w1")
                nc.scalar.activation(w1_t[:], e_t[:], AT.Exp)
                w2_t = ew.tile([P, GC, adw], f32, tag="w2")
                nc.scalar.activation(w2_t[:], e_t[:], AT.Exp, scale=c.NEG)
                nc.vector.tensor_tensor(
                    out=gt[:, :, wcol: wcol + adw], in0=w1_t[:],
                    in1=w2_t[:], op=OP.max)
                # msg: h *= w (broadcast over ch)
                nc.vector.tensor_tensor(
                    out=gt[:, :, 0: hc].rearrange(
                        "p c (h x) -> p c h x", h=heads),
                    in0=gt[:, :, 0: hc].rearrange(
                        "p c (h x) -> p c h x", h=heads),
                    in1=gt[:, :, wcol: wcol + adw
                           ].to_broadcast([P, GC, adw, ch]),
                    op=OP.mult)
                # fused aggregation + denominator matmuls
                out_ps = ep2.tile([P, rhsw], f32, space="PSUM", tag="ops")
                for k in range(GC):
                    nc.tensor.matmul(
                        out_ps[:], lhsT=S_g[:, k, :],
                        rhs=gt[:, k, 0: rhsw],
                        start=(k == 0), stop=(k == GC - 1))
                epilogue(w, out_ps)

            for w in range(c.NW + LEAD):
                if w < c.NW:
                    issue_lo(w)
                if w >= LEAD:
                    run_window(w - LEAD)

        # ---------------- phase 2: L1 edge windows (+fused L2 matmul) -------
        with tc.tile_pool(name="eg", bufs=6) as eg, \
             tc.tile_pool(name="emeta", bufs=1) as emeta, \
             tc.tile_pool(name="ew", bufs=3) as ew, \
             tc.tile_pool(name="eS", bufs=3) as eS, \
             tc.tile_pool(name="ep1", bufs=2, space="PSUM") as ep1, \
             tc.tile_pool(name="ep2", bufs=2, space="PSUM") as ep2, \
             tc.tile_pool(name="ep3", bufs=2, space="PSUM") as ep3:
            idx_t = emeta.tile([P, TOTC * (P // 16)], mybir.dt.int16)
            nc.sync.dma_start(idx_t[:], idx_d[:])
            cnt_t = emeta.tile([1, 2 * c.NW], mybir.dt.int32)
            nc.sync.dma_start(cnt_t[:], cnts_d[:])

            def epi1(w, out_ps):
                s_sb = ew.tile([P, c.HEADS], f32, tag="ssb")
                nc.vector.tensor_scalar_add(
                    s_sb[:], out_ps[:, c.HID: c.HID + c.HEADS], 1e-16)
                rs = ew.tile([P, c.HEADS], f32, tag="rs")
                nc.vector.reciprocal(rs[:], s_sb[:])
                z = ew.tile([P, c.HID], f32, tag="z")
                nc.vector.tensor_tensor(
                    out=z[:].rearrange("p (h x) -> p h x", h=c.HEADS),
                    in0=out_ps[:, 0: c.HID].rearrange(
                        "p (h x) -> p h x", h=c.HEADS),
                    in1=rs[:].to_broadcast([P, c.HEADS, c.CH]), op=OP.mult)
                # act1' = elu(z)+1 = exp(-relu(-z)) + relu(z)
                r1 = ew.tile([P, c.HID], f32, tag="r1")
                nc.scalar.activation(r1[:], z[:], AT.Relu, scale=-1.0)
                em = ew.tile([P, c.HID], f32, tag="em")
                nc.scalar.activation(em[:], r1[:], AT.Exp, scale=-1.0)
                r2 = ew.tile([P, c.HID], f32, tag="r2")
                nc.scalar.activation(r2[:], z[:], AT.Relu)
                a1p = ew.tile([P, c.HID], f32, tag="a1p")
                nc.vector.tensor_tensor(
                    out=a1p[:], in0=em[:], in1=r2[:], op=OP.add)
                # fused L2 matmul: h2 = act1'@W2e - colsum(W2e)
                h2ps = ep3.tile([P, c.T2], f32, space="PSUM", tag="h2ps")
                for half in range(c.KC2):
                    tp = ep3.tile([P, P], f32, space="PSUM", tag="tp")
                    nc.tensor.transpose(
                        out=tp[:], in_=a1p[:, half * P: (half + 1) * P],
                        identity=ident[:])
                    a1c = ew.tile([P, P], bf16, tag="a1c")
                    nc.scalar.copy(a1c[:], tp[:])
                    nc.tensor.matmul(
                        h2ps[:], lhsT=a1c[:], rhs=w2s[half][:],
                        start=(half == 0), stop=(half == c.KC2 - 1))
                h2sb = ew.tile([P, c.T2], f32, tag="h2sb")
                nc.vector.tensor_tensor(
                    out=h2sb[:], in0=h2ps[:], in1=negrow_t[:], op=OP.add)
                h2bf = ew.tile([P, c.T2], bf16, tag="h2bf")
                nc.scalar.copy(h2bf[:], h2sb[:])
                nc.vector.tensor_copy(
                    out=h2own_all[:, w, :], in_=h2bf[:, 0: c.W2C])
                nc.sync.dma_start(h2_bounce[w * P: (w + 1) * P, :], h2bf[:])
                if w in ag_marks:
                    ag_chunk(ag_marks[w], h2_bounce, h2_tab_lo, h2_tab_hi)

            edge_phase((eg, ew, eS, ep1, ep2), h_tab_lo, h_tab_hi, ad_all,
                       hown_all, c.HID + c.HEADS,
                       c.HEADS, c.HEADS, c.CH, c.T1, c.HID,
                       c.HID + c.HEADS, epi1)

        # ---------------- phase 4: L2 edge windows ----------------
        with tc.tile_pool(name="eg2", bufs=6) as eg, \
             tc.tile_pool(name="emeta2", bufs=1) as emeta, \
             tc.tile_pool(name="ew2", bufs=3) as ew, \
             tc.tile_pool(name="eS2", bufs=3) as eS, \
             tc.tile_pool(name="ep12", bufs=3, space="PSUM") as ep1, \
             tc.tile_pool(name="ep22", bufs=3, space="PSUM") as ep2:
            idx_t = emeta.tile([P, TOTC * (P // 16)], mybir.dt.int16)
            nc.sync.dma_start(idx_t[:], idx_d[:])
            cnt_t = emeta.tile([1, 2 * c.NW], mybir.dt.int32)
            nc.sync.dma_start(cnt_t[:], cnts_d[:])

            def epi2(w, out_ps):
                wd = min(P, c.DPC - w * P)
                s_sb = ew.tile([P, 1], f32, tag="ssb2")
                nc.vector.tensor_scalar_add(
                    s_sb[:], out_ps[:, c.NCLS: c.NCLS + 1], 1e-16)
                rs = ew.tile([P, 1], f32, tag="rs2")
                nc.vector.reciprocal(rs[:], s_sb[:])
                z = ew.tile([P, c.NCLS], f32, tag="z2")
                nc.vector.tensor_tensor(
                    out=z[:], in0=out_ps[:, 0: c.NCLS],
                    in1=rs[:].to_broadcast([P, c.NCLS]), op=OP.mult)
                if not meta.get("b2_zero"):
                    nc.vector.tensor_tensor(
                        out=z[:], in0=z[:], in1=b2r_t[:], op=OP.add)
                nc.sync.dma_start(
                    out_d[w * P: w * P + wd, :], z[0: wd, :])

            edge_phase((eg, ew, eS, ep1, ep2), h2_tab_lo, h2_tab_hi,
                       h2own_all[:, :, c.NCLS + 1: c.NCLS + 2],
                       h2own_all, c.NCLS + 1,
                       1, 1, c.NCLS, c.T2, c.NCLS, c.NCLS + 1, epi2)

    nc.compile()
    return nc


_CACHE = {}
TRACE = False
LAST = None


def kernel(**inputs):
    global LAST
    from concourse.bass_utils import run_bass_kernel_spmd

    cfg = Cfg()
    x = np.asarray(inputs["x"], np.float32)
    ei = np.asarray(inputs["edge_index"], np.int64)
    meta, in_maps = preprocess(
        cfg, x, ei, inputs["W1"], inputs["att_src1"], inputs["att_dst1"],
        inputs["b1"], inputs["W2"], inputs["att_src2"], inputs["att_dst2"],
        inputs["b2"])
    key = (meta["TOTC"], tuple(meta["LC"]), tuple(meta["HC"]),
           meta["b2_zero"])
    if key not in _CACHE:
        _CACHE[key] = build_program(cfg, meta)
    nc = _CACHE[key]
    res = run_bass_kernel_spmd(nc, in_maps, core_ids=list(range(cfg.NCORES)),
                               trace=TRACE)
    LAST = res
    out = np.concatenate([res.results[co]["out"] for co in range(cfg.NCORES)],
                         axis=0)
    return out.astype(np.float32)


# revision 42
# speedup vs baseline: 1.0851x; 1.0851x over previous
"""GAT (2-layer, 8-head) Trainium2 Bass kernel, 8-core SPMD. v2.

Strategy (dst-sharded edge partition, superwindows of 128 dsts):
- Host: append self-loops, shard edges by dst range (6250 dsts/core), bucket
  into 49 superwindows of 128 dsts, split each window's edges by src<32768
  (lo/hi for int16 dma_gather indexing). Sections padded to 128-edge chunks
  with SPMD-uniform (max-over-cores) chunk counts; pad slots carry idx=-1
  (skipped by the gather HW when num_idxs_reg = per-core valid count) and
  all-zero rows/cols in the one-hot S/ST matrices.
- Device phase 1: sharded matmul xT @ W1ext -> h rows
  [h(256) | a_src(8) | a_dst(8) | pad] bf16; a_dst slice kept in SBUF
  (ad_all); AllGather of the h table is CHUNKED (5 pieces) and overlaps the
  matmul tiles.
- Phase 2 (per superwindow): dma_gather h[src] rows (768B, Q7-bound, back to
  back); ed = ST-chunk @ a_dst matmuls; e = a_src[src]+ed (vector);
  w = max(exp(e), exp(0.2e)) (scalar engine exps, vector max) written into
  the gathered tile's a_dst cols; msg = h*w; one fused matmul per chunk
  accumulates aggregation + softmax denominators in PSUM; epilogue computes
  act1'=elu+1 via exp/relu on the scalar engine, transposes it, and fuses
  the layer-2 matmul (W2ext, with the -1 correction folded as a replicated
  constant row) -> h2 rows [h2(10)|as2|ad2|0...] written to h2_bounce;
  AllGather-2 is chunked behind the window loop.
- Phase 4: same edge pipeline with 1 head, 10 channels on 256B h2 rows ->
  final [6250, 10] fp32 slice per core; host concatenates.
"""
import os
import sys
from contextlib import ExitStack

for _p in ("/opt/trn_rl_repo", os.path.expanduser("~/.axon_site/_ro/trn_rl_repo")):
    if os.path.isdir(_p) and _p not in sys.path:
        sys.path.insert(0, _p)

import numpy as np
import ml_dtypes

P = 128


class Cfg:
    def __init__(self):
        self.N, self.F, self.HEADS, self.CH, self.NCLS = 50000, 767, 8, 32, 10
        self.NCORES, self.SPLIT, self.NEG = 8, 30000, 0.2
        self.HID = self.HEADS * self.CH            # 256
        self.DPC = self.N // self.NCORES           # 6250 dsts per core
        self.NW = (self.DPC + P - 1) // P          # 49 superwindows
        self.DPCP = self.NW * P                    # 6272 padded rows/core
        self.FP = (self.F + P - 1) // P * P        # 768
        self.KC1 = self.FP // P                    # 6
        self.W1C = self.HID + 2 * self.HEADS       # 272 used cols
        self.T1 = 384                              # 768B table rows
        self.KC2 = self.HID // P                   # 2
        self.W2C = self.NCLS + 2                   # 12 used cols
        self.T2 = 128                              # 256B table rows
        self.AGC = 5                               # AllGather chunks
        assert self.DPC % self.AGC == 0
        self.AGR = self.DPC // self.AGC            # 1250 rows/core/chunk


def _wrap_idxs(vals, nslots):
    """int16 vals (len<=nslots) -> [128, nslots/16] wrapped, pads=-1."""
    cols = nslots // 16
    arr = np.full((16, cols), -1, dtype=np.int16)
    n = len(vals)
    if n:
        j = np.arange(n)
        arr[j % 16, j // 16] = vals
    return np.tile(arr, (8, 1))


def preprocess(cfg, x, edge_index, W1, att_src1, att_dst1, b1, W2, att_src2,
               att_dst2, b2):
    c = cfg
    N = c.N
    # self-loops are handled locally in the epilogues, not as edges
    src = np.asarray(edge_index[0]).astype(np.int64)
    dst = np.asarray(edge_index[1]).astype(np.int64)

    # --- weight prep (param folding only) ---
    W1 = np.asarray(W1, np.float32)
    a_s1 = np.asarray(att_src1, np.float32)
    a_d1 = np.asarray(att_dst1, np.float32)
    W1e = np.zeros((c.FP, c.T1), np.float32)
    W1e[: c.F, : c.HID] = W1
    for h in range(c.HEADS):
        blk = W1[:, h * c.CH: (h + 1) * c.CH]
        W1e[: c.F, c.HID + h] = blk @ a_s1[h]
        W1e[: c.F, c.HID + c.HEADS + h] = blk @ a_d1[h]
    W2 = np.asarray(W2, np.float32)
    W2e = np.zeros((c.HID, c.T2), np.float32)
    W2e[:, : c.NCLS] = W2
    W2e[:, c.NCLS] = W2 @ np.asarray(att_src2, np.float32)[0]
    W2e[:, c.NCLS + 1] = W2 @ np.asarray(att_dst2, np.float32)[0]
    # act1 is stored as elu+1; fold the -1 row correction into a replicated
    # constant added to every h2 row.
    negrow = -W2e.sum(axis=0)                       # [T2]
    negrow_rep = np.tile(negrow[None, :], (P, 1)).astype(np.float32)

    # --- per-core edge bucketing (numpy group-by) ---
    core = dst // c.DPC
    dloc = dst - core * c.DPC
    win = dloc // P
    dcol = dloc % P
    # Table rows are laid out (ag_chunk, core, row) so each chunked
    # AllGather writes a contiguous slab; remap gather indices to match.
    core_s = src // c.DPC
    rr = src - core_s * c.DPC
    psrc = ((rr // c.AGR) * (c.AGR * c.NCORES) + core_s * c.AGR
            + rr % c.AGR)
    src = psrc
    is_hi = (src >= c.SPLIT).astype(np.int64)
    sec = ((core * c.NW + win) * 2 + is_hi)
    order = np.argsort(sec, kind="stable")
    sec_s = sec[order]
    src_s = src[order]
    dcol_s = dcol[order]
    nsec = c.NCORES * c.NW * 2
    bounds = np.searchsorted(sec_s, np.arange(nsec + 1))
    cnts = (bounds[1:] - bounds[:-1]).reshape(c.NCORES, c.NW, 2)

    def nch(n):
        return max(1, (int(n) + P - 1) // P)

    LC = [nch(cnts[:, w, 0].max()) for w in range(c.NW)]
    HC = [nch(cnts[:, w, 1].max()) for w in range(c.NW)]
    # +1 self-loop identity chunk per window (filled on-device, no gather)
    TOTC = sum(LC) + sum(HC) + c.NW
    sec_c0 = []                                    # chunk offset per (w, kind)
    off = 0
    for w in range(c.NW):
        sec_c0.append((off, off + LC[w]))
        off += LC[w] + HC[w] + 1
    meta = {"LC": LC, "HC": HC, "TOTC": TOTC, "sec_c0": sec_c0,
            "b2_zero": not np.any(np.asarray(b2))}

    in_maps = []
    xf = np.asarray(x, np.float32)
    ar = np.arange(P)
    for co in range(c.NCORES):
        idx_parts = []
        cnt_arr = np.zeros((1, 2 * c.NW), np.int32)
        dstc = np.full((P, TOTC), -1, np.int32)    # slot -> dst col (-1=pad)
        for w in range(c.NW):
            for kind in (0, 1):
                s = (co * c.NW + w) * 2 + kind
                b0, b1_ = bounds[s], bounds[s + 1]
                vals = src_s[b0:b1_] - (c.SPLIT if kind else 0)
                dcs = dcol_s[b0:b1_]
                nck = LC[w] if kind == 0 else HC[w]
                cstart = sec_c0[w][kind]
                n = b1_ - b0
                if n == 0:
                    # keep >=1 valid idx so the gather is never empty
                    vals = np.zeros(1, np.int64)
                    dcs = np.full(1, -2, np.int64)  # no S entry
                    n = 1
                j = np.arange(n)
                dstc[j % P, cstart + j // P] = np.where(dcs >= 0, dcs, -1)
                idx_parts.append(_wrap_idxs(vals.astype(np.int16), nck * P))
                cnt_arr[0, 2 * w + kind] = n
            # self-loop identity chunk (no idx cols; filled on-device)
            wd = min(P, c.DPC - w * P)
            sc = sec_c0[w][0] + LC[w] + HC[w]
            dstc[0:wd, sc] = np.arange(wd)
            idx_parts.append(np.full((P, P // 16), -1, np.int16))
        idx_np = np.concatenate(idx_parts, axis=1)
        assert idx_np.shape[1] == TOTC * (P // 16)

        S_host = (dstc[:, :, None] == ar[None, None, :])
        ST_host = (dstc.T[None, :, :] == ar[:, None, None])   # [j, chunk, e]
        S_host = S_host.astype(ml_dtypes.float8_e4m3).reshape(P, TOTC * P)
        ST_host = ST_host.astype(ml_dtypes.float8_e4m3).reshape(P, TOTC * P)

        xT = np.zeros((c.FP, c.DPCP), ml_dtypes.bfloat16)
        xs = xf[co * c.DPC: (co + 1) * c.DPC]
        xT[: c.F, : c.DPC] = xs.T.astype(ml_dtypes.bfloat16)

        in_maps.append({
            "xT": xT,
            "W1e": W1e.astype(ml_dtypes.bfloat16),
            "W2e": W2e.astype(ml_dtypes.bfloat16),
            "negrow": negrow_rep,
            "idx": idx_np,
            "cnts": cnt_arr,
            "Sh": S_host,
            "STh": ST_host,
            "b2r": np.tile(np.asarray(b2, np.float32)[None, :], (P, 1)),
        })
    return meta, in_maps


def build_program(cfg, meta):
    import concourse.bacc as bacc
    import concourse.mybir as mybir
    import concourse.tile as tile
    from concourse.library_config import mlp
    from concourse.masks import make_identity

    c = cfg
    f32, bf16 = mybir.dt.float32, mybir.dt.bfloat16
    fp8 = mybir.dt.float8e4
    AT = mybir.ActivationFunctionType
    OP = mybir.AluOpType

    nc = bacc.Bacc("TRN2", target_bir_lowering=False, debug=False,
                   num_devices=c.NCORES, num_swdge_queues=4)
    TOTC = meta["TOTC"]
    LC, HC, sec_c0 = meta["LC"], meta["HC"], meta["sec_c0"]
    GCs = [LC[w] + HC[w] + 1 for w in range(c.NW)]

    xT_d = nc.dram_tensor("xT", [c.FP, c.DPCP], bf16, kind="ExternalInput")
    W1e_d = nc.dram_tensor("W1e", [c.FP, c.T1], bf16, kind="ExternalInput")
    W2e_d = nc.dram_tensor("W2e", [c.HID, c.T2], bf16, kind="ExternalInput")
    negrow_d = nc.dram_tensor("negrow", [P, c.T2], f32, kind="ExternalInput")
    idx_d = nc.dram_tensor("idx", [P, TOTC * (P // 16)], mybir.dt.int16,
                           kind="ExternalInput")
    cnts_d = nc.dram_tensor("cnts", [1, 2 * c.NW], mybir.dt.int32,
                            kind="ExternalInput")
    Sh_d = nc.dram_tensor("Sh", [P, TOTC * P], fp8, kind="ExternalInput")
    STh_d = nc.dram_tensor("STh", [P, TOTC * P], fp8, kind="ExternalInput")
    b2r_d = nc.dram_tensor("b2r", [P, c.NCLS], f32, kind="ExternalInput")
    out_d = nc.dram_tensor("out", [c.DPC, c.NCLS], f32, kind="ExternalOutput")

    _shared = "Shared" if c.NCORES > 4 else "Local"
    AGR = c.AGR                                  # rows per AllGather chunk
    AGW = AGR * c.NCORES                         # global rows per chunk
    LO_CH = c.SPLIT // AGW                       # chunks in the lo table (3)
    assert c.SPLIT % AGW == 0
    h_bounce = nc.dram_tensor("h_bounce", [c.DPCP, c.T1], bf16, kind="Internal")
    h_tab_lo = nc.dram_tensor("h_tab_lo", [c.SPLIT, c.T1], bf16,
                              kind="Internal", addr_space=_shared)
    h_tab_hi = nc.dram_tensor("h_tab_hi", [c.N - c.SPLIT, c.T1], bf16,
                              kind="Internal", addr_space=_shared)
    h2_bounce = nc.dram_tensor("h2_bounce", [c.DPCP, c.T2], bf16,
                               kind="Internal")
    h2_tab_lo = nc.dram_tensor("h2_tab_lo", [c.SPLIT, c.T2], bf16,
                               kind="Internal", addr_space=_shared)
    h2_tab_hi = nc.dram_tensor("h2_tab_hi", [c.N - c.SPLIT, c.T2], bf16,
                               kind="Internal", addr_space=_shared)

    ag_marks = {}
    for ci in range(c.AGC):
        ag_marks[min(((ci + 1) * AGR + P - 1) // P, c.NW) - 1] = ci
    groups8 = [list(range(c.NCORES))]

    def ag_chunk(ci, bounce, tab_lo, tab_hi):
        r0, r1 = ci * AGR, (ci + 1) * AGR
        tab, g0 = ((tab_lo, ci * AGW) if ci < LO_CH
                   else (tab_hi, ci * AGW - c.SPLIT))
        nc.gpsimd.collective_compute(
            "AllGather", OP.bypass, replica_groups=groups8,
            ins=[bounce.ap()[r0:r1, :]],
            outs=[tab.ap()[g0: g0 + AGW, :]])

    with ExitStack() as stack:
        tc = stack.enter_context(tile.TileContext(nc))
        cpool = stack.enter_context(tc.tile_pool(name="consts", bufs=1))
        nc.gpsimd.load_library(mlp)

        ident = cpool.tile([P, P], f32)
        make_identity(nc, ident[:])
        b2r_t = cpool.tile([P, c.NCLS], f32)
        nc.sync.dma_start(b2r_t[:], b2r_d[:])
        negrow_t = cpool.tile([P, c.T2], f32)
        nc.sync.dma_start(negrow_t[:], negrow_d[:])
        w2s = []
        for k in range(c.KC2):
            t = cpool.tile([P, c.T2], bf16, tag=f"w2s{k}")
            nc.sync.dma_start(t[:], W2e_d[k * P: (k + 1) * P, :])
            w2s.append(t)
        ad_all = cpool.tile([P, c.NW, c.HEADS], bf16, tag="ad_all")
        hown_all = cpool.tile([P, c.NW, c.HID + c.HEADS], bf16, tag="hown")
        h2own_all = cpool.tile([P, c.NW, c.W2C], bf16, tag="h2own")

        # ---------------- phase 1: L1 matmul (sharded rows) ----------------
        with tc.tile_pool(name="mm1", bufs=1) as mm1, \
             tc.tile_pool(name="mm1w", bufs=3) as mm1w, \
             tc.tile_pool(name="mm1p", bufs=2, space="PSUM") as mm1p:
            w1s = []
            for k in range(c.KC1):
                t = mm1.tile([P, c.T1], bf16, tag=f"w1s{k}")
                nc.sync.dma_start(t[:], W1e_d[k * P: (k + 1) * P, :])
                w1s.append(t)
            # xT loaded in column halves so matmuls start after ~half the load
            RH0 = (c.NW + 1) // 2                  # tiles in first half
            HW0 = RH0 * P
            xts = [[None, None] for _ in range(c.KC1)]
            for half in range(2):
                cw = HW0 if half == 0 else c.DPCP - HW0
                for k in range(c.KC1):
                    t = mm1.tile([P, cw], bf16, tag=f"xts{k}_{half}")
                    nc.sync.dma_start(
                        t[:], xT_d[k * P: (k + 1) * P,
                                   half * HW0: half * HW0 + cw])
                    xts[k][half] = t
            for r in range(c.NW):
                half, rh = (0, r) if r < RH0 else (1, r - RH0)
                ps = mm1p.tile([P, c.T1], f32, space="PSUM", tag="mmps")
                for k in range(c.KC1):
                    nc.tensor.matmul(
                        ps[:], lhsT=xts[k][half][:, rh * P: (rh + 1) * P],
                        rhs=w1s[k][:], start=(k == 0), stop=(k == c.KC1 - 1))
                hsb = mm1w.tile([P, c.T1], bf16, tag="hsb")
                nc.scalar.copy(hsb[:], ps[:])
                nc.vector.tensor_copy(
                    out=ad_all[:, r, :],
                    in_=hsb[:, c.HID + c.HEADS: c.HID + 2 * c.HEADS])
                nc.vector.tensor_copy(
                    out=hown_all[:, r, :],
                    in_=hsb[:, 0: c.HID + c.HEADS])
                nc.sync.dma_start(h_bounce[r * P: (r + 1) * P, :], hsb[:])
                if r in ag_marks:
                    ag_chunk(ag_marks[r], h_bounce, h_tab_lo, h_tab_hi)

        # ---------------- shared edge-window pipeline ----------------
        LEAD = 3

        def edge_phase(pools, tab_lo, tab_hi, ad_tile, own_tile, owncols,
                       adw, heads, ch, gelem, wcol, rhsw, epilogue):
            eg, ew, eS, ep1, ep2 = pools
            hc = heads * ch
            GCmax = max(GCs)
            qn = [0]
            cnt_reg = nc.gpsimd.alloc_register(f"cnt_reg_{id(epilogue)}")
            gts = {}

            def gather(gt, sec_n, sec_i, col_off, out_off, in_ap):
                nidx = sec_n * P
                nc.gpsimd.reg_load(
                    cnt_reg, cnt_t[0:1, sec_i: sec_i + 1])
                nc.gpsimd.dma_gather(
                    gt[:, out_off: out_off + sec_n, :], in_ap,
                    idx_t[:, col_off * (P // 16):
                          (col_off + sec_n) * (P // 16)],
                    nidx, cnt_reg, gelem, single_packet=False,
                    queue_num=qn[0] % 4)
                qn[0] += 1

            def issue_lo(w):
                GC = GCs[w]
                c0 = sec_c0[w][0]
                gtf = eg.tile([P, GCmax, gelem], bf16, tag="gt")
                if w < 2:
                    nc.vector.memset(gtf[:], 0.0)
                else:
                    nc.vector.memset(gtf[:, :, hc: hc + adw], 0.0)
                gt = gtf[:, 0:GC, :]
                gts[w] = gt
                gather(gt, LC[w], 2 * w, c0, 0, tab_lo.ap())

            def run_window(w):
                GC = GCs[w]
                c0 = sec_c0[w][0]
                gt = gts.pop(w)
                gather(gt, HC[w], 2 * w + 1, c0 + LC[w], LC[w],
                       tab_hi.ap())
                # self-loop chunk: own rows copied locally, S/ST = identity
                # (scalar engine: DVE writes into gt stall against in-flight
                # gather DMA writes to the same buffers)
                nc.scalar.copy(
                    gt[:, GC - 1, 0: owncols],
                    own_tile[:, w, 0: owncols])
                S_g = eS.tile([P, GC, P], fp8, tag="Sg")
                nc.sync.dma_start(S_g[:], Sh_d[:, c0 * P: (c0 + GC) * P])
                ST_g = eS.tile([P, GC, P], fp8, tag="STg")
                nc.sync.dma_start(ST_g[:], STh_d[:, c0 * P: (c0 + GC) * P])

                # ed = a_dst broadcast per edge slot
                ed_ps = ep1.tile([P, GC, adw], f32, space="PSUM", tag="edps")
                for k in range(GC):
                    nc.tensor.matmul(
                        ed_ps[:, k, :], lhsT=ST_g[:, k, :],
                        rhs=ad_tile[:, w, :], start=True, stop=True)
                # e = a_src + ed; w = max(exp(e), exp(0.2 e))
                e_t = ew.tile([P, GC, adw], f32, tag="e")
                nc.vector.tensor_tensor(
                    out=e_t[:], in0=gt[:, :, hc: hc + adw],
                    in1=ed_ps[:], op=OP.add)
                w1_t = ew.tile([P, GC, adw], f32, tag="w1")
                nc.scalar.activation(w1_t[:], e_t[:], AT.Exp)
                w2_t = ew.tile([P, GC, adw], f32, tag="w2")
                nc.scalar.activation(w2_t[:], e_t[:], AT.Exp, scale=c.NEG)
                nc.vector.tensor_tensor(
                    out=gt[:, :, wcol: wcol + adw], in0=w1_t[:],
                    in1=w2_t[:], op=OP.max)
                # msg: h *= w (broadcast over ch)
                nc.vector.tensor_tensor(
                    out=gt[:, :, 0: hc].rearrange(
                        "p c (h x) -> p c h x", h=heads),
                    in0=gt[:, :, 0: hc].rearrange(
                        "p c (h x) -> p c h x", h=heads),
                    in1=gt[:, :, wcol: wcol + adw
                           ].to_broadcast([P, GC, adw, ch]),
                    op=OP.mult)
                # fused aggregation + denominator matmuls
                out_ps = ep2.tile([P, rhsw], f32, space="PSUM", tag="ops")
                for k in range(GC):
                    nc.tensor.matmul(
                        out_ps[:], lhsT=S_g[:, k, :],
                        rhs=gt[:, k, 0: rhsw],
                        start=(k == 0), stop=(k == GC - 1))
                epilogue(w, out_ps)

            for w in range(c.NW + LEAD):
                if w < c.NW:
                    issue_lo(w)
                if w >= LEAD:
                    run_window(w - LEAD)

        # ---------------- phase 2: L1 edge windows (+fused L2 matmul) -------
        with tc.tile_pool(name="eg", bufs=6) as eg, \
             tc.tile_pool(name="emeta", bufs=1) as emeta, \
             tc.tile_pool(name="ew", bufs=3) as ew, \
             tc.tile_pool(name="eS", bufs=3) as eS, \
             tc.tile_pool(name="ep1", bufs=2, space="PSUM") as ep1, \
             tc.tile_pool(name="ep2", bufs=2, space="PSUM") as ep2, \
             tc.tile_pool(name="ep3", bufs=2, space="PSUM") as ep3:
            idx_t = emeta.tile([P, TOTC * (P // 16)], mybir.dt.int16)
            nc.sync.dma_start(idx_t[:], idx_d[:])
            cnt_t = emeta.tile([1, 2 * c.NW], mybir.dt.int32)
            nc.sync.dma_start(cnt_t[:], cnts_d[:])

            def epi1(w, out_ps):
                s_sb = ew.tile([P, c.HEADS], f32, tag="ssb")
                nc.vector.tensor_scalar_add(
                    s_sb[:], out_ps[:, c.HID: c.HID + c.HEADS], 1e-16)
                rs = ew.tile([P, c.HEADS], f32, tag="rs")
                nc.vector.reciprocal(rs[:], s_sb[:])
                z = ew.tile([P, c.HID], f32, tag="z")
                nc.vector.tensor_tensor(
                    out=z[:].rearrange("p (h x) -> p h x", h=c.HEADS),
                    in0=out_ps[:, 0: c.HID].rearrange(
                        "p (h x) -> p h x", h=c.HEADS),
                    in1=rs[:].to_broadcast([P, c.HEADS, c.CH]), op=OP.mult)
                # act1' = elu(z)+1 = exp(-relu(-z)) + relu(z)
                r1 = ew.tile([P, c.HID], f32, tag="r1")
                nc.scalar.activation(r1[:], z[:], AT.Relu, scale=-1.0)
                em = ew.tile([P, c.HID], f32, tag="em")
                nc.scalar.activation(em[:], r1[:], AT.Exp, scale=-1.0)
                r2 = ew.tile([P, c.HID], f32, tag="r2")
                nc.scalar.activation(r2[:], z[:], AT.Relu)
                a1p = ew.tile([P, c.HID], f32, tag="a1p")
                nc.vector.tensor_tensor(
                    out=a1p[:], in0=em[:], in1=r2[:], op=OP.add)
                # fused L2 matmul: h2 = act1'@W2e - colsum(W2e)
                h2ps = ep3.tile([P, c.T2], f32, space="PSUM", tag="h2ps")
                for half in range(c.KC2):
                    tp = ep3.tile([P, P], f32, space="PSUM", tag="tp")
                    nc.tensor.transpose(
                        out=tp[:], in_=a1p[:, half * P: (half + 1) * P],
                        identity=ident[:])
                    a1c = ew.tile([P, P], bf16, tag="a1c")
                    nc.scalar.copy(a1c[:], tp[:])
                    nc.tensor.matmul(
                        h2ps[:], lhsT=a1c[:], rhs=w2s[half][:],
                        start=(half == 0), stop=(half == c.KC2 - 1))
                h2sb = ew.tile([P, c.T2], f32, tag="h2sb")
                nc.vector.tensor_tensor(
                    out=h2sb[:], in0=h2ps[:], in1=negrow_t[:], op=OP.add)
                h2bf = ew.tile([P, c.T2], bf16, tag="h2bf")
                nc.scalar.copy(h2bf[:], h2sb[:])
                nc.vector.tensor_copy(
                    out=h2own_all[:, w, :], in_=h2bf[:, 0: c.W2C])
                nc.sync.dma_start(h2_bounce[w * P: (w + 1) * P, :], h2bf[:])
                if w in ag_marks:
                    ag_chunk(ag_marks[w], h2_bounce, h2_tab_lo, h2_tab_hi)

            edge_phase((eg, ew, eS, ep1, ep2), h_tab_lo, h_tab_hi, ad_all,
                       hown_all, c.HID + c.HEADS,
                       c.HEADS, c.HEADS, c.CH, c.T1, c.HID,
                       c.HID + c.HEADS, epi1)

        # ---------------- phase 4: L2 edge windows ----------------
        with tc.tile_pool(name="eg2", bufs=6) as eg, \
             tc.tile_pool(name="emeta2", bufs=1) as emeta, \
             tc.tile_pool(name="ew2", bufs=3) as ew, \
             tc.tile_pool(name="eS2", bufs=3) as eS, \
             tc.tile_pool(name="ep12", bufs=3, space="PSUM") as ep1, \
             tc.tile_pool(name="ep22", bufs=3, space="PSUM") as ep2:
            idx_t = emeta.tile([P, TOTC * (P // 16)], mybir.dt.int16)
            nc.sync.dma_start(idx_t[:], idx_d[:])
            cnt_t = emeta.tile([1, 2 * c.NW], mybir.dt.int32)
            nc.sync.dma_start(cnt_t[:], cnts_d[:])

            def epi2(w, out_ps):
                wd = min(P, c.DPC - w * P)
                s_sb = ew.tile([P, 1], f32, tag="ssb2")
                nc.vector.tensor_scalar_add(
                    s_sb[:], out_ps[:, c.NCLS: c.NCLS + 1], 1e-16)
                rs = ew.tile([P, 1], f32, tag="rs2")
                nc.vector.reciprocal(rs[:], s_sb[:])
                z = ew.tile([P, c.NCLS], f32, tag="z2")
                nc.vector.tensor_tensor(
                    out=z[:], in0=out_ps[:, 0: c.NCLS],
                    in1=rs[:].to_broadcast([P, c.NCLS]), op=OP.mult)
                if not meta.get("b2_zero"):
                    nc.vector.tensor_tensor(
                        out=z[:], in0=z[:], in1=b2r_t[:], op=OP.add)
                nc.sync.dma_start(
                    out_d[w * P: w * P + wd, :], z[0: wd, :])

            edge_phase((eg, ew, eS, ep1, ep2), h2_tab_lo, h2_tab_hi,
                       h2own_all[:, :, c.NCLS + 1: c.NCLS + 2],
                       h2own_all, c.NCLS + 1,
                       1, 1, c.NCLS, c.T2, c.NCLS, c.NCLS + 1, epi2)

    nc.compile()
    return nc


_CACHE = {}
TRACE = False
LAST = None


def kernel(**inputs):
    global LAST
    from concourse.bass_utils import run_bass_kernel_spmd

    cfg = Cfg()
    x = np.asarray(inputs["x"], np.float32)
    ei = np.asarray(inputs["edge_index"], np.int64)
    meta, in_maps = preprocess(
        cfg, x, ei, inputs["W1"], inputs["att_src1"], inputs["att_dst1"],
        inputs["b1"], inputs["W2"], inputs["att_src2"], inputs["att_dst2"],
        inputs["b2"])
    key = (meta["TOTC"], tuple(meta["LC"]), tuple(meta["HC"]),
           meta["b2_zero"])
    if key not in _CACHE:
        _CACHE[key] = build_program(cfg, meta)
    nc = _CACHE[key]
    res = run_bass_kernel_spmd(nc, in_maps, core_ids=list(range(cfg.NCORES)),
                               trace=TRACE)
    LAST = res
    out = np.concatenate([res.results[co]["out"] for co in range(cfg.NCORES)],
                         axis=0)
    return out.astype(np.float32)
